# revision 62
# baseline (speedup 1.0000x reference)
"""Trainium2 Bass kernel for GQA attention (B=4, S=2048, HID=896, H=14, KV=2, D=64).

Sharding: 8 cores = 4 batches x 2 KV-head groups. Core c handles batch c//2,
query heads [g*7, (g+1)*7) with g = c%2 (exactly one KV head per core thanks to
GQA group structure). Each core computes its 448-channel slice of attn output
and the partial output projection y_g = ao_g @ Wo[g*448:(g+1)*448, :]; the host
sums the two partials per batch.

Engine budget per core (cost-model): ACT is the wall (~140us of exp - the only
engine with transcendentals), PE ~157us of matmul streaming, DVE ~85us, Pool
(GPSIMD) ~55us of offloaded copies/masks/broadcasts. The kernel keeps ACT and
PE dense by interleaving projection/output-projection/transpose work between
attention chunks (fills) so neither engine ever waits long.

Work placement:
  PE   projections, PE-transposes, scoresT = kT.T @ qT, attn@[v|1] (ones row
       gives softmax denominators for free), y = ao @ Wo.
  ACT  exp only (pairs of k-tiles share one [128,1024] PSUM + one exp).
  DVE  RoPE (rotate-half form, f16 2x mode), PSUM evacuations of transposes +
       attn rows, reciprocal of denominators, final normalize muls (batched
       per chunk via strided 3D APs).
  Pool kv/q/y PSUM evacuations, causal triu masks on diagonal tiles,
       partition-broadcast of denominator rows (replaces a DRAM roundtrip).

RoPE is computed in rotate-half (GPT-NeoX) layout: host permutes Wq/Wk columns
per head (even channels first, then odd), making all DVE access patterns
contiguous 32-element blocks -> 2x packed-f16 mode. Scores are invariant to a
shared channel permutation of q and k; v/Wo untouched.

The causal mask input is never loaded: exp(-1e9 + s) == 0.0 exactly in fp32,
so structural masking (k-tiles <= diagonal; diagonal tiles exp'd then masked
with a triangular 0/1 multiply on Pool) matches the reference bit-for-bit.

mm_dt selects the matmul dtype: float16 (1 PE cycle/row) default.
"""
import math
import os
import numpy as np

import concourse.bass as bass
import concourse.mybir as mybir
import concourse.tile as tile
from concourse import bacc
from concourse.masks import make_identity

F32 = mybir.dt.float32
F32R = mybir.dt.float32r
F16 = mybir.dt.float16
AF = mybir.ActivationFunctionType

B, S, HID = 4, 2048, 896
H, KV, D = 14, 2, 64
HL = H // KV          # 7 local query heads per core
GD = HL * D           # 448 local channels
KCH = HID // 128      # 7 contraction chunks
N_CORES = 8


def _bc7(ap_small):
    """[128, 64] cos/sin slice -> broadcast over the 7 heads: [128, 7, 64]."""
    return bass.AP(
        tensor=ap_small.tensor,
        offset=ap_small.offset,
        ap=[list(ap_small.ap[0]), [0, HL], list(ap_small.ap[1])],
    )


def _swap_halves(ap3):
    """[128, n, 64] k/q slice -> halves swapped: [128, n, 2, 32] reading
    (second, first) 32-block of each 64-channel group. Keeps inner step=1 so
    DVE stays in 2x packed-f16 mode."""
    *outer, last = ap3.ap
    assert last[0] == 1 and last[1] == 64
    return bass.AP(
        tensor=ap3.tensor,
        offset=ap3.offset + 32,
        ap=[list(d) for d in outer] + [[-32, 2], [1, 32]],
    )


def _hsel(tile_ap, h0, step, n, fd_off=0, fd_len=None, pcount=64):
    """araw/rbF-style [64|65, HL, fd] tile -> heads h0, h0+step, ... (n of
    them), free sub-range [fd_off, fd_off+fd_len), as a 3D AP."""
    p_dim, h_dim, f_dim = tile_ap.ap
    if fd_len is None:
        fd_len = f_dim[1] - fd_off
    return bass.AP(
        tensor=tile_ap.tensor,
        offset=tile_ap.offset + h0 * h_dim[0] + fd_off * f_dim[0],
        ap=[[p_dim[0], pcount], [h_dim[0] * step, n], [f_dim[0], fd_len]],
    )


def build(s=S, mm_dt=None, reps=1):
    if mm_dt is None:
        mm_dt = MM_DT
    ST = s // 128           # s-tiles
    QC = s // 512           # q chunks (also the number of super-blocks)
    TPB = ST // QC          # s-tiles per super-block (4)
    nc = bacc.Bacc("TRN2", target_bir_lowering=False, debug=False,
                   num_devices=N_CORES)

    xT = nc.dram_tensor("xT", [HID, s], mm_dt, kind="ExternalInput").ap()
    wq = nc.dram_tensor("wq", [HID, GD], mm_dt, kind="ExternalInput").ap()
    wkv = nc.dram_tensor("wkv", [HID, 128], mm_dt, kind="ExternalInput").ap()
    wo = nc.dram_tensor("wo", [GD, HID], mm_dt, kind="ExternalInput").ap()
    cs2 = nc.dram_tensor("cs2", [128, ST, 128], F16, kind="ExternalInput").ap()
    triu = nc.dram_tensor("triu", [128, 128], mm_dt, kind="ExternalInput").ap()
    y = nc.dram_tensor("y", [s, HID], F32, kind="ExternalOutput").ap()

    with tile.TileContext(nc) as tc:
        with (
            tc.tile_pool(name="wp", bufs=1) as wp,
            tc.tile_pool(name="per", bufs=1) as per,
            tc.tile_pool(name="tmp", bufs=2) as tmp,
            tc.tile_pool(name="expp", bufs=1) as expp,
            tc.tile_pool(name="rb", bufs=1) as rb,
            tc.tile_pool(name="xp", bufs=1) as xp,
        ):
            # ---- rep-invariant loads, ordered + split so the first s-tile's
            # inputs land first (~4us to first matmul): per-k-chunk wq, wkv,
            # the first 128 columns of x, tables, then the rest of x, then
            # wo (not needed until the first output projection ~40us in).
            # All stay resident in SBUF across reps. ----
            wq_sb = wp.tile([128, KCH, GD], mm_dt, tag="wq", name="wq")
            wqr = wq.rearrange("(k p) m -> p k m", p=128)
            nc.sync.dma_start(out=wq_sb[:, 0:4, :], in_=wqr[:, 0:4, :])
            xT_sb = xp.tile([128, KCH, s], mm_dt, tag="xT", name="xT")
            xr = xT.rearrange("(k p) m -> p k m", p=128)
            nc.sync.dma_start(out=xT_sb[:, :, 0:512], in_=xr[:, :, 0:512])
            nc.sync.dma_start(out=wq_sb[:, 4:KCH, :], in_=wqr[:, 4:KCH, :])
            cs_sb = wp.tile([128, ST, 128], F16, tag="cs", name="cs")
            nc.sync.dma_start(out=cs_sb[:], in_=cs2)
            triu_sb = wp.tile([128, 128], mm_dt, tag="triu", name="triu")
            nc.sync.dma_start(out=triu_sb[:], in_=triu)
            wkv_sb = wp.tile([128, KCH, 128], mm_dt, tag="wkv", name="wkv")
            nc.sync.dma_start(out=wkv_sb[:], in_=wkv.rearrange("(k p) m -> p k m", p=128))
            nc.sync.dma_start(out=xT_sb[:, :, 512:1024], in_=xr[:, :, 512:1024])
            nc.sync.dma_start(out=xT_sb[:, :, 1024:s], in_=xr[:, :, 1024:s])
            wo_sb = wp.tile([128, 4, HID], mm_dt, tag="wo", name="wo")
            for cc in range(4):
                w = 128 if cc < 3 else 64
                nc.sync.dma_start(out=wo_sb[0:w, cc, :], in_=wo[cc * 128:cc * 128 + w, :])

            idn = wp.tile([128, 128], F32, tag="idn", name="idn")
            make_identity(nc, idn[:])
            idn_r = wp.tile([128, 128], mm_dt, tag="idnr", name="idnr")
            nc.vector.tensor_copy(idn_r[:], idn[:])
            idn_mm = idn_r[:]
            # preload the exp table set while ACT is otherwise idle, so the
            # ~1.3us ACT_TABLE_LOAD is off the first real exp's critical path
            warm = wp.tile([1, 2], F32, tag="warm", name="warm")
            nc.vector.memset(warm[:], 0.0)
            nc.scalar.activation(out=warm[:], in_=warm[:], func=AF.Exp)
            # warm the PE clock (HAM un-throttles after ~3.4us of sustained
            # matmul activity) with dummy matmuls while the input DMAs are in
            # flight, so the first projections run at full rate
            with tc.tile_pool(name="psW", bufs=1, space="PSUM") as psW:
                warm_ps = psW.tile([128, 128], F32, tag="wps", name="wps")
                for _ in range(42):
                    nc.tensor.matmul(warm_ps[:], idn_r[:], idn_r[:],
                                     start=True, stop=True)

            # qT / aoT: qT in head-pair chunks (chunk j holds heads 2j, 2j+1);
            # aoT unified [128, 4, s] so the normalize muls batch across heads
            # with one strided AP per partition half.
            q_pair = [per.tile([128 if j < 3 else 64, s], mm_dt,
                               tag=f"qp{j}", name=f"qp{j}") for j in range(4)]
            aoT = per.tile([128, 4, s], mm_dt, tag="ao", name="ao")
            kT2 = per.tile([128, s], mm_dt, tag="kT2", name="kT2")
            # merged kv staging: [k(64) | v(64) | ones(1)] per s-tile
            kv_all = per.tile([128, ST, 129], mm_dt, tag="kv_all", name="kv_all")
            if mm_dt == F16:
                nc.vector.memset(
                    kv_all[:, :, 128:129].bitcast(mybir.dt.uint16), 0x3C00)
            else:
                nc.vector.memset(kv_all[:, :, 128:129].bitcast(F32), 1.0)

            def _body():
                LOOKP = 2
                with tc.tile_pool(name="psA", bufs=1, space="PSUM") as psA, \
                     tc.tile_pool(name="psQ", bufs=1, space="PSUM") as psQ:
                    # t-slot provider: one PSUM bank of 8 rotating [128,128]
                    # f16 quarter-slots for PE transposes.
                    tstate = {"slot": 0}
                    t_bank = psA.tile([128, 8, 128], mm_dt, tag="t", name="t")

                    def _tslot():
                        sl = tstate["slot"]
                        tstate["slot"] = (sl + 1) % 8
                        return t_bank[:, sl, :]

                    def _emit_qtrans(st, q_rot, pre=False):
                        # PSUM evacuations: ACT during the preamble (it is
                        # idle until the first exp), DVE in steady state
                        # (ACT is the kernel's critical engine there)
                        for cc in range(4):
                            w = 128 if cc < 3 else 64
                            t_ps = _tslot()[0:w, :]
                            nc.tensor.transpose(t_ps, q_rot[:, cc * 128:cc * 128 + w],
                                                idn_mm)
                            dst = q_pair[cc][:, st * 128:(st + 1) * 128]
                            if pre:
                                nc.scalar.copy(out=dst, in_=t_ps)
                            else:
                                nc.vector.tensor_copy(dst, t_ps)

                    pend_a = []
                    pend_mul = []
                    attn_state = {"psB": None}

                    def emit_out_st(st):
                        # one s-tile of y = ao @ Wo, emitted as a fill INSIDE
                        # the next block's attention. The y accumulator
                        # borrows a psB rotation slot ([128,1024] = same
                        # shape), so no extra PSUM banks are needed; the exp
                        # pipeline simply runs one buffer short for the ~1.5us
                        # the wo matmuls occupy the PE anyway.
                        y_ps = attn_state["psB"].tile([128, 1024], F32,
                                                      tag="sp", name="sp", bufs=2)
                        for cc in range(4):
                            w = 128 if cc < 3 else 64
                            lhsT = aoT[0:w, cc, st * 128:(st + 1) * 128]
                            nc.tensor.matmul(y_ps[:, 0:512], lhsT,
                                             wo_sb[0:w, cc, 0:512],
                                             start=(cc == 0), stop=(cc == 3))
                            nc.tensor.matmul(y_ps[:, 512:896], lhsT,
                                             wo_sb[0:w, cc, 512:896],
                                             start=(cc == 0), stop=(cc == 3))
                        y_sb = tmp.tile([128, HID], F32, tag="ysb", name="ysb")
                        nc.vector.tensor_copy(y_sb[:], y_ps[:, 0:896])
                        nc.sync.dma_start(out=y[st * 128:(st + 1) * 128, :],
                                          in_=y_sb[:])

                    def emit_proj(st, qpool=None, qbufs=1, pre=False):
                        if qpool is not None:
                            q_ps = qpool.tile([128, GD], F32, tag="q", name="q",
                                              bufs=qbufs)[:]
                            kv_ps = qpool.tile([128, 128], F32, tag="kv",
                                               name="kv", bufs=qbufs)[:]
                        else:
                            q_ps = psQ.tile([128, GD], F32, tag="q",
                                            name="q", bufs=1)[:]
                            kv_ps = psQ.tile([128, 128], F32, tag="kv",
                                             name="kv", bufs=1)[:]
                        for kc in range(KCH):
                            lhsT = xT_sb[:, kc, st * 128:(st + 1) * 128]
                            nc.tensor.matmul(q_ps, lhsT, wq_sb[:, kc, :],
                                             start=(kc == 0), stop=(kc == KCH - 1))
                        for kc in range(KCH):
                            lhsT = xT_sb[:, kc, st * 128:(st + 1) * 128]
                            nc.tensor.matmul(kv_ps, lhsT, wkv_sb[:, kc, :],
                                             start=(kc == 0), stop=(kc == KCH - 1))
                        # k, v staged raw (RoPE'd in place per block); q cast
                        # to f16 for the 2x-mode RoPE chain
                        q_sb = tmp.tile([128, GD], F16, tag="qsb", name="qsb",
                                        bufs=2)
                        if pre:
                            nc.scalar.copy(out=kv_all[:, st, 0:128], in_=kv_ps)
                            nc.scalar.copy(out=q_sb[:], in_=q_ps)
                        else:
                            nc.vector.tensor_copy(kv_all[:, st, 0:128], kv_ps)
                            nc.vector.tensor_copy(q_sb[:], q_ps)
                        # rotate-half RoPE, all-f16 contiguous 32-blocks (2x):
                        #   qrot[:32] = q[:32]*c - q[32:]*s
                        #   qrot[32:] = q[:32]*s + q[32:]*c
                        # via qrot = q*cos2 + swap(q)*sin2, sin2 = [-s | s].
                        qv = q_sb[:].rearrange("p (h d) -> p h d", d=D)
                        cb = _bc7(cs_sb[:, st, 0:64])
                        sb_ = _bc7(cs_sb[:, st, 64:128])
                        t1 = tmp.tile([128, HL, D], F16, tag="t1", name="t1", bufs=2)
                        t2 = tmp.tile([128, HL, D], F16, tag="t2", name="t2", bufs=2)
                        nc.vector.tensor_mul(t1[:], qv, cb)
                        nc.vector.tensor_mul(t2[:], _swap_halves(qv), sb_)
                        q_rot = tmp.tile([128, GD], mm_dt, tag="qrot", name="qrot",
                                         bufs=3)
                        qrv = q_rot[:].rearrange("p (h d) -> p h d", d=D)
                        nc.vector.tensor_add(qrv, t1[:], t2[:])
                        # transposes for the PREVIOUS s-tile go after this
                        # tile's projections so PE never waits on the RoPE DVE
                        pend_a.append((st, q_rot))
                        if len(pend_a) > 1:
                            _emit_qtrans(*pend_a.pop(0), pre=pre)

                    def emit_krope(st_lo, st_hi):
                        n = st_hi - st_lo
                        kv3 = kv_all[:, st_lo:st_hi, 0:64]
                        cs = cs_sb[:, st_lo:st_hi, 0:64]
                        ss = cs_sb[:, st_lo:st_hi, 64:128]
                        k1 = tmp.tile([128, TPB, 64], F16, tag="k1", name="k1", bufs=2)
                        k2 = tmp.tile([128, TPB, 64], F16, tag="k2", name="k2", bufs=2)
                        nc.vector.tensor_mul(k1[:, 0:n, :], kv3, cs)
                        nc.vector.tensor_mul(k2[:, 0:n, :], _swap_halves(kv3), ss)
                        nc.vector.tensor_add(kv3, k1[:, 0:n, :], k2[:, 0:n, :])

                    def emit_ktrans(st_lo, st_hi, pre=False):
                        for st in range(st_lo, st_hi):
                            t_ps = _tslot()[0:64, :]
                            nc.tensor.transpose(t_ps, kv_all[:, st, 0:64], idn_mm)
                            lo = kT2[0:64, st * 128:(st + 1) * 128]
                            hi = kT2[64:128, st * 128:(st + 1) * 128]
                            if pre:
                                nc.scalar.copy(out=lo, in_=t_ps)
                                nc.scalar.copy(out=hi, in_=t_ps)
                            else:
                                nc.vector.tensor_copy(lo, t_ps)
                                nc.vector.tensor_copy(hi, t_ps)

                    def emit_attn(qc, fills=()):
                        # k-tiles in pairs sharing a [128,1024] psum + one exp;
                        # LOOKP pairs in flight so PE stays ahead of ACT.
                        # Unwritten psum regions of partial (diagonal) tiles
                        # hold stale garbage whose exp is never consumed.
                        araw = rb.tile([65, HL, 512], F32, tag="araw",
                                       name="araw", bufs=2)
                        rbF = rb.tile([64, HL, 512], F32, tag="rbF",
                                      name="rbF", bufs=2)
                        with tc.tile_pool(name="psB", bufs=1, space="PSUM") as psB, \
                             tc.tile_pool(name="psO", bufs=1, space="PSUM") as psO:
                            attn_state["psB"] = psB
                            nkt = 4 * (qc + 1)
                            npair = (nkt + 1) // 2
                            fill_iter = iter(fills)

                            def emit_pair(h, pi, qc=qc, nkt=nkt):
                                half = (h % 2) * 64
                                qsrc = q_pair[h // 2]
                                s_ps = psB.tile([128, 1024], F32, tag="sp",
                                                name="sp", bufs=2)
                                ex = expp.tile([128, 1024], mm_dt, tag="ex",
                                               name="ex", bufs=4)
                                info = []
                                for j in (0, 1):
                                    kt = 2 * pi + j
                                    if kt >= nkt:
                                        break
                                    rrel = kt - 4 * qc
                                    off = 128 * rrel if rrel >= 0 else 0
                                    N = 512 - off
                                    nc.tensor.matmul(
                                        s_ps[:, 512 * j + off:512 * (j + 1)],
                                        kT2[half:half + 64, kt * 128:(kt + 1) * 128],
                                        qsrc[half:half + 64,
                                             qc * 512 + off:(qc + 1) * 512],
                                        start=True, stop=True)
                                    info.append((kt, 512 * j + off, off, N, rrel))
                                # exp over each contiguous written run (a
                                # diagonal second tile leaves an unwritten gap)
                                runs = []
                                for kt, base, off, N, rrel in info:
                                    if runs and runs[-1][1] == base:
                                        runs[-1][1] = base + N
                                    else:
                                        runs.append([base, base + N])
                                for lo, hi in runs:
                                    nc.scalar.activation(out=ex[:, lo:hi],
                                                         in_=s_ps[:, lo:hi],
                                                         func=AF.Exp)
                                for kt, base, off, N, rrel in info:
                                    if rrel >= 0:
                                        nc.vector.tensor_mul(
                                            ex[:, base:base + 128],
                                            ex[:, base:base + 128], triu_sb[:])
                                return ex, info

                            # flat (head, pair) task stream: the scores/exp
                            # lookahead crosses head boundaries, so ACT keeps
                            # an exp backlog through evacuations and fills.
                            tasks = [(h, pi) for h in range(HL)
                                     for pi in range(npair)]
                            LOOKA = LOOKP + 1
                            pend = {}
                            o_ref = {}
                            for i in range(min(LOOKA, len(tasks))):
                                pend[tasks[i]] = emit_pair(*tasks[i])
                            for i, (h, pi) in enumerate(tasks):
                                if i + LOOKA < len(tasks):
                                    pend[tasks[i + LOOKA]] = emit_pair(*tasks[i + LOOKA])
                                if pi == 0:
                                    o_ref[h] = psO.tile([65, 512], F32, tag="o",
                                                        name="o", bufs=1)
                                o_ps = o_ref[h]
                                ex, info = pend.pop((h, pi))
                                for kt, base, off, N, rrel in info:
                                    nc.tensor.matmul(
                                        o_ps[:, off:512], kv_all[:, kt, 64:129],
                                        ex[:, base:base + N],
                                        start=(kt == 0), stop=(kt == nkt - 1))
                                if pi == npair - 1:
                                    # ---- per-head-chunk softmax norm ----
                                    # attn rows + den row staged in one copy
                                    # (frees the psum fast); den row then
                                    # partition-broadcast on Pool (SBUF-only
                                    # engine); fast-reciprocal of the
                                    # broadcast rows; the normalize multiply
                                    # is DEFERRED a block and batched across
                                    # heads.
                                    nc.vector.tensor_copy(araw[:, h, :],
                                                          o_ps[:])
                                    # den row staged to a partition-0 tile:
                                    # the Pool broadcast reads partition 0
                                    den_sb = tmp.tile([1, 512], F32, tag="den",
                                                      name="den", bufs=4)
                                    nc.vector.tensor_copy(den_sb[:],
                                                          o_ps[64:65, :])
                                    denB = tmp.tile([64, 512], F32, tag="denB",
                                                    name="denB", bufs=4)
                                    nc.gpsimd.partition_broadcast(denB[:],
                                                                  den_sb[:])
                                    nc.vector.reciprocal_approx_fast(
                                        rbF[:, h, :], denB[:])
                                    # block 0: defer fills past head 1 so the
                                    # list scheduler can't run next-block
                                    # projections ahead of this block's first
                                    # scores (which wait on the preamble's
                                    # transpose chain)
                                    if qc > 0 or h >= 2:
                                        for _ in range(2):
                                            f = next(fill_iter, None)
                                            if f is not None:
                                                f()
                        pend_mul.append((qc, araw[:], rbF[:]))

                    def _emit_muls(qc, araw_ap, rbF_ap, fd_off=0, fd_len=512):
                        # heads 0,2,4,6 -> aoT partitions 0:64, chunks 0..3;
                        # heads 1,3,5 -> partitions 64:128, chunks 0..2.
                        ao = aoT[:]
                        dst0 = bass.AP(
                            tensor=ao.tensor,
                            offset=ao.offset + qc * 512 + fd_off,
                            ap=[[ao.ap[0][0], 64], [ao.ap[1][0], 4], [1, fd_len]])
                        nc.vector.tensor_mul(
                            dst0, _hsel(araw_ap, 0, 2, 4, fd_off, fd_len),
                            _hsel(rbF_ap, 0, 2, 4, fd_off, fd_len))
                        dst1 = bass.AP(
                            tensor=ao.tensor,
                            offset=ao.offset + 64 * ao.ap[0][0] + qc * 512 + fd_off,
                            ap=[[ao.ap[0][0], 64], [ao.ap[1][0], 3], [1, fd_len]])
                        nc.vector.tensor_mul(
                            dst1, _hsel(araw_ap, 1, 2, 3, fd_off, fd_len),
                            _hsel(rbF_ap, 1, 2, 3, fd_off, fd_len))

                    def _flush_muls():
                        for args in pend_mul:
                            _emit_muls(*args)
                        pend_mul.clear()

                    def emit_out_one(st, pool, tail=False):
                        y_ps = pool.tile([128, 1024], F32, tag="y", name="y")
                        for cc in range(4):
                            w = 128 if cc < 3 else 64
                            lhsT = aoT[0:w, cc, st * 128:(st + 1) * 128]
                            nc.tensor.matmul(y_ps[:, 0:512], lhsT,
                                             wo_sb[0:w, cc, 0:512],
                                             start=(cc == 0), stop=(cc == 3))
                            nc.tensor.matmul(y_ps[:, 512:896], lhsT,
                                             wo_sb[0:w, cc, 512:896],
                                             start=(cc == 0), stop=(cc == 3))
                        y_sb = tmp.tile([128, HID], F32, tag="ysb", name="ysb")
                        if tail:
                            nc.scalar.copy(out=y_sb[:], in_=y_ps[:, 0:896])
                        else:
                            nc.vector.tensor_copy(y_sb[:], y_ps[:, 0:896])
                        nc.sync.dma_start(out=y[st * 128:(st + 1) * 128, :],
                                          in_=y_sb[:])

                    # fine-grained interleave: the next block's projections,
                    # k-RoPE, transposes and the previous block's deferred
                    # normalization muls are emitted BETWEEN attention chunks,
                    # so the PE queue always holds independent matmuls where
                    # the exp's fixed per-instruction overhead would otherwise
                    # idle it.
                    def _mk_flushn():
                        n0 = len(pend_mul)

                        def _f():
                            for args in pend_mul[:n0]:
                                _emit_muls(*args)
                            del pend_mul[:n0]
                        return _f

                    def _mk_fills(nb):
                        # fills for emit_attn(nb-1): flush block nb-2's
                        # normalize muls, then its 4 wo s-tiles interleaved
                        # with block nb's 4 projections, then k-RoPE and the
                        # transposes for block nb.
                        # steady-state fill evacuations stay on DVE; ACT only
                        # absorbs them in the preamble/tail where it idles.
                        # Transposes are spread across per-tile kq fills so
                        # the 6 packed t-slots never back up.
                        def _kq(st):
                            def _f():
                                emit_krope(st, st + 1)
                                while pend_a and pend_a[0][0] <= st:
                                    _emit_qtrans(*pend_a.pop(0))
                                emit_ktrans(st, st + 1)
                            return _f

                        fills = [_mk_flushn()] if nb >= 2 else []
                        for i in range(TPB):
                            if nb >= 2:
                                fills.append(lambda t=(nb - 2) * TPB + i: emit_out_st(t))
                            if nb < QC:
                                fills.append(lambda t=nb * TPB + i: emit_proj(t))
                                if i >= 1:
                                    fills.append(_kq(nb * TPB + i - 1))
                        if nb < QC:
                            fills.append(_kq(nb * TPB + TPB - 1))
                        return fills

                    # preamble: per-s-tile proj -> RoPE -> transposes, fully
                    # pipelined so the first scores/exp start as early as
                    # possible (the first chunk's scores need ALL 4 s-tiles'
                    # qT, so each tile's transposes overlap the next tile's
                    # projection matmuls). A dedicated double-buffered PSUM
                    # pool (banks are otherwise idle before attention starts)
                    # keeps the projections back-to-back on the PE instead of
                    # serializing on each tile's evacuation.
                    with tc.tile_pool(name="psP", bufs=1, space="PSUM") as psP:
                        emit_proj(0, psP, 2, pre=True)
                        for st in range(1, TPB):
                            emit_proj(st, psP, 2, pre=True)
                            emit_krope(st - 1, st)
                            while pend_a and pend_a[0][0] < st:
                                _emit_qtrans(*pend_a.pop(0), pre=True)
                            emit_ktrans(st - 1, st, pre=True)
                        emit_krope(TPB - 1, TPB)
                        while pend_a:
                            _emit_qtrans(*pend_a.pop(0), pre=True)
                        emit_ktrans(TPB - 1, TPB, pre=True)
                    for blk in range(QC):
                        emit_attn(blk, _mk_fills(blk + 1))
                    # tail: the last block's normalize muls are split per
                    # s-tile and interleaved with its wo projections, so the
                    # PE starts on y as soon as the first 128 queries are
                    # normalized instead of after the full batched muls.
                    (qcL, arawL, rbFL) = pend_mul.pop()
                    assert not pend_mul
                    with tc.tile_pool(name="psC", bufs=2, space="PSUM") as psC:
                        for i in range(TPB):
                            _emit_muls(qcL, arawL, rbFL, i * 128, 128)
                            emit_out_one(qcL * TPB + i, psC, tail=True)

            if reps > 1:
                with tc.For_i(0, reps, 1):
                    _body()
            else:
                _body()

    nc.compile()
    return nc


# ---------------------------------------------------------------------------
# host-side sharding + execution
# ---------------------------------------------------------------------------

def round_f32r(a):
    """Round fp32 array to fp32r (RNE to 11 mantissa bits)."""
    b = np.ascontiguousarray(a, dtype=np.float32).view(np.uint32)
    lsb = (b >> np.uint32(12)) & np.uint32(1)
    r = ((b + np.uint32(0x7FF) + lsb) & np.uint32(0xFFFFF000))
    return r.view(np.float32)


MM_DT = {"f32r": F32R, "f16": F16, "f32": F32}[os.environ.get("MM_DT", "f16")]


def _cvt(a, mm_dt):
    if mm_dt == F16:
        return np.ascontiguousarray(np.asarray(a, dtype=np.float32)).astype(np.float16)
    if mm_dt == F32R:
        return round_f32r(a)
    return np.ascontiguousarray(a, dtype=np.float32)


# rotate-half channel permutation within each 64-channel head: evens first,
# then odds. Applied to Wq/Wk columns; scores are invariant to a shared
# permutation of q and k channels, and v/Wo are untouched.
_PERM64 = np.concatenate([np.arange(0, 64, 2), np.arange(1, 64, 2)])


def _perm_heads(w, nheads):
    w = np.asarray(w)
    cols = np.concatenate([h * 64 + _PERM64 for h in range(nheads)])
    return w[:, cols]


def make_in_maps(x, freqs_cos, freqs_sin, Wq, Wk, Wv, Wo, s=S, mm_dt=None):
    if mm_dt is None:
        mm_dt = MM_DT
    ST = s // 128
    scale = 1.0 / math.sqrt(D)
    cosr = np.ascontiguousarray(
        np.asarray(freqs_cos).reshape(ST, 128, 32).transpose(1, 0, 2)).astype(np.float32)
    sinr = np.ascontiguousarray(
        np.asarray(freqs_sin).reshape(ST, 128, 32).transpose(1, 0, 2)).astype(np.float32)
    cs2 = np.concatenate([cosr, cosr, -sinr, sinr], axis=2).astype(np.float16)
    triu = _cvt(np.triu(np.ones((128, 128), dtype=np.float32)), mm_dt)
    in_maps = []
    for c in range(N_CORES):
        b, g = c // 2, c % 2
        wq_g = _perm_heads(np.asarray(Wq)[:, g * GD:(g + 1) * GD] * scale, HL)
        wk_g = _perm_heads(np.asarray(Wk)[:, g * D:(g + 1) * D], 1)
        in_maps.append({
            "xT": _cvt(np.asarray(x)[b].T, mm_dt),
            "wq": _cvt(wq_g, mm_dt),
            "wkv": _cvt(np.concatenate(
                [wk_g, np.asarray(Wv)[:, g * D:(g + 1) * D]], axis=1), mm_dt),
            "wo": _cvt(np.asarray(Wo)[g * GD:(g + 1) * GD, :], mm_dt),
            "cs2": cs2, "triu": triu,
        })
    return in_maps


_RUNNER = None


class _Runner:
    """Minimal SPMD executor over axon PJRT (self-contained copy)."""

    def __init__(self, nc, n_cores):
        import jax
        from jax.sharding import Mesh, PartitionSpec, NamedSharding
        from jax.experimental.shard_map import shard_map
        from concourse.bass2jax import (_bass_exec_p, install_neuronx_cc_hook,
                                        partition_id_tensor)
        install_neuronx_cc_hook()
        self.jax = jax
        self.n_cores = n_cores
        partition_name = (nc.partition_id_tensor.name
                          if nc.partition_id_tensor else None)
        in_names, out_names, out_avals = [], [], []
        for alloc in nc.m.functions[0].allocations:
            if not isinstance(alloc, mybir.MemoryLocationSet):
                continue
            name = alloc.memorylocations[0].name
            if alloc.kind == "ExternalInput":
                if name != partition_name:
                    in_names.append(name)
            elif alloc.kind == "ExternalOutput":
                out_names.append(name)
                out_avals.append(jax.core.ShapedArray(
                    tuple(alloc.tensor_shape), mybir.dt.np(alloc.dtype)))
        self.in_names, self.out_names, self.out_avals = in_names, out_names, out_avals
        n_params, n_outs = len(in_names), len(out_avals)
        all_names = in_names + out_names
        if partition_name is not None:
            all_names.append(partition_name)

        def _body(*args):
            operands = list(args)
            if partition_name is not None:
                operands.append(partition_id_tensor())
            return tuple(_bass_exec_p.bind(
                *operands, out_avals=tuple(out_avals), in_names=tuple(all_names),
                out_names=tuple(out_names), lowering_input_output_aliases=(),
                sim_require_finite=False, sim_require_nnan=False, nc=nc))

        devices = jax.devices()[:n_cores]
        self.mesh = Mesh(np.asarray(devices), ("core",))
        self.sharding = NamedSharding(self.mesh, PartitionSpec("core"))
        in_specs = (PartitionSpec("core"),) * (n_params + n_outs)
        out_specs = (PartitionSpec("core"),) * n_outs
        self.fn = jax.jit(
            shard_map(_body, mesh=self.mesh, in_specs=in_specs,
                      out_specs=out_specs, check_rep=False),
            donate_argnums=tuple(range(n_params, n_params + n_outs)),
            keep_unused=True)
        zshapes = [(n_cores * a.shape[0], *a.shape[1:]) for a in out_avals]
        zdtypes = [a.dtype for a in out_avals]
        self.make_zeros = jax.jit(
            lambda: tuple(jax.numpy.zeros(sh, dt)
                          for sh, dt in zip(zshapes, zdtypes)),
            out_shardings=tuple(self.sharding for _ in zshapes))

    def prep(self, in_maps):
        return [self.jax.device_put(
            np.concatenate([np.asarray(in_maps[c][n]) for c in range(self.n_cores)],
                           axis=0), self.sharding)
            for n in self.in_names]

    def run(self, dev_in):
        return self.fn(*dev_in, *self.make_zeros())

    def split(self, outs):
        res = []
        for c in range(self.n_cores):
            res.append({n: np.asarray(outs[i]).reshape(
                self.n_cores, *self.out_avals[i].shape)[c]
                for i, n in enumerate(self.out_names)})
        return res


def get_runner():
    global _RUNNER
    if _RUNNER is None:
        _RUNNER = _Runner(build(), N_CORES)
    return _RUNNER


def kernel(x, freqs_cos, freqs_sin, mask, Wq, Wk, Wv, Wo):
    x = np.asarray(x, dtype=np.float32)
    in_maps = make_in_maps(np.asarray(x), np.asarray(freqs_cos),
                           np.asarray(freqs_sin), np.asarray(Wq),
                           np.asarray(Wk), np.asarray(Wv), np.asarray(Wo))
    r = get_runner()
    outs = r.run(r.prep(in_maps))
    res = r.split(outs)
    out = np.empty((B, S, HID), dtype=np.float32)
    for b in range(B):
        out[b] = res[2 * b]["y"] + res[2 * b + 1]["y"]
    return out


# revision 63
# speedup vs baseline: 1.0729x; 1.0729x over previous
"""Trainium2 Bass kernel for GQA attention (B=4, S=2048, HID=896, H=14, KV=2, D=64).

Sharding: 8 cores = 4 batches x 2 KV-head groups. Core c handles batch c//2,
query heads [g*7, (g+1)*7) with g = c%2 (exactly one KV head per core thanks to
GQA group structure). Each core computes its 448-channel slice of attn output
and the partial output projection y_g = ao_g @ Wo[g*448:(g+1)*448, :]; the host
sums the two partials per batch.

Engine budget per core (cost-model): ACT is the wall (~140us of exp - the only
engine with transcendentals), PE ~157us of matmul streaming, DVE ~85us, Pool
(GPSIMD) ~55us of offloaded copies/masks/broadcasts. The kernel keeps ACT and
PE dense by interleaving projection/output-projection/transpose work between
attention chunks (fills) so neither engine ever waits long.

Work placement (NOTE: GPSIMD/Pool cannot access PSUM on real HW - walrus
rejects it even though CoreSim's cost model accepts it):
  PE   projections, PE-transposes, scoresT = kT.T @ qT, attn@[v|1] (ones row
       gives softmax denominators for free), y = ao @ Wo, plus ~42 warm-up
       matmuls so the HAM clock gate is released before the first projection.
  ACT  exp (pairs of k-tiles share one [128,1024] PSUM + one exp; the exp
       table set is preloaded at t=0), preamble/tail PSUM evacuations while
       it would otherwise idle.
  DVE  RoPE (rotate-half form, f16 2x mode), steady-state PSUM evacuations
       (transposes, kv/q, attn rows + den, y), triu masks, reciprocal of
       denominators, normalize muls (batched across heads via strided 3D
       APs, deferred one block).
  Pool partition-broadcast of denominator rows from a partition-0 SBUF tile
       (replaces the baseline's DRAM roundtrip).

RoPE is computed in rotate-half (GPT-NeoX) layout: host permutes Wq/Wk columns
per head (even channels first, then odd), making all DVE access patterns
contiguous 32-element blocks -> 2x packed-f16 mode. Scores are invariant to a
shared channel permutation of q and k; v/Wo untouched.

The causal mask input is never loaded: exp(-1e9 + s) == 0.0 exactly in fp32,
so structural masking (k-tiles <= diagonal; diagonal tiles exp'd then masked
with a triangular 0/1 multiply on Pool) matches the reference bit-for-bit.

mm_dt selects the matmul dtype: float16 (1 PE cycle/row) default.
"""
import math
import os
import numpy as np

import concourse.bass as bass
import concourse.mybir as mybir
import concourse.tile as tile
from concourse import bacc
from concourse.masks import make_identity

F32 = mybir.dt.float32
F32R = mybir.dt.float32r
F16 = mybir.dt.float16
AF = mybir.ActivationFunctionType

B, S, HID = 4, 2048, 896
H, KV, D = 14, 2, 64
HL = H // KV          # 7 local query heads per core
GD = HL * D           # 448 local channels
KCH = HID // 128      # 7 contraction chunks
N_CORES = 8


def _bc7(ap_small):
    """[128, 64] cos/sin slice -> broadcast over the 7 heads: [128, 7, 64]."""
    return bass.AP(
        tensor=ap_small.tensor,
        offset=ap_small.offset,
        ap=[list(ap_small.ap[0]), [0, HL], list(ap_small.ap[1])],
    )


def _swap_halves(ap3):
    """[128, n, 64] k/q slice -> halves swapped: [128, n, 2, 32] reading
    (second, first) 32-block of each 64-channel group. Keeps inner step=1 so
    DVE stays in 2x packed-f16 mode."""
    *outer, last = ap3.ap
    assert last[0] == 1 and last[1] == 64
    return bass.AP(
        tensor=ap3.tensor,
        offset=ap3.offset + 32,
        ap=[list(d) for d in outer] + [[-32, 2], [1, 32]],
    )


def _hsel(tile_ap, h0, step, n, fd_off=0, fd_len=None, pcount=64):
    """araw/rbF-style [64|65, HL, fd] tile -> heads h0, h0+step, ... (n of
    them), free sub-range [fd_off, fd_off+fd_len), as a 3D AP."""
    p_dim, h_dim, f_dim = tile_ap.ap
    if fd_len is None:
        fd_len = f_dim[1] - fd_off
    return bass.AP(
        tensor=tile_ap.tensor,
        offset=tile_ap.offset + h0 * h_dim[0] + fd_off * f_dim[0],
        ap=[[p_dim[0], pcount], [h_dim[0] * step, n], [f_dim[0], fd_len]],
    )


def build(s=S, mm_dt=None, reps=1):
    if mm_dt is None:
        mm_dt = MM_DT
    ST = s // 128           # s-tiles
    QC = s // 512           # q chunks (also the number of super-blocks)
    TPB = ST // QC          # s-tiles per super-block (4)
    nc = bacc.Bacc("TRN2", target_bir_lowering=False, debug=False,
                   num_devices=N_CORES)

    xT = nc.dram_tensor("xT", [HID, s], mm_dt, kind="ExternalInput").ap()
    wq = nc.dram_tensor("wq", [HID, GD], mm_dt, kind="ExternalInput").ap()
    wkv = nc.dram_tensor("wkv", [HID, 128], mm_dt, kind="ExternalInput").ap()
    wo = nc.dram_tensor("wo", [GD, HID], mm_dt, kind="ExternalInput").ap()
    cs2 = nc.dram_tensor("cs2", [128, ST, 128], F16, kind="ExternalInput").ap()
    triu = nc.dram_tensor("triu", [128, 128], mm_dt, kind="ExternalInput").ap()
    y = nc.dram_tensor("y", [s, HID], F32, kind="ExternalOutput").ap()

    with tile.TileContext(nc) as tc:
        with (
            tc.tile_pool(name="wp", bufs=1) as wp,
            tc.tile_pool(name="per", bufs=1) as per,
            tc.tile_pool(name="tmp", bufs=2) as tmp,
            tc.tile_pool(name="expp", bufs=1) as expp,
            tc.tile_pool(name="rb", bufs=1) as rb,
            tc.tile_pool(name="xp", bufs=1) as xp,
        ):
            # ---- rep-invariant loads, ordered + split so the first s-tile's
            # inputs land first (~4us to first matmul): per-k-chunk wq, wkv,
            # the first 128 columns of x, tables, then the rest of x, then
            # wo (not needed until the first output projection ~40us in).
            # All stay resident in SBUF across reps. ----
            wq_sb = wp.tile([128, KCH, GD], mm_dt, tag="wq", name="wq")
            wqr = wq.rearrange("(k p) m -> p k m", p=128)
            nc.sync.dma_start(out=wq_sb[:, 0:4, :], in_=wqr[:, 0:4, :])
            xT_sb = xp.tile([128, KCH, s], mm_dt, tag="xT", name="xT")
            xr = xT.rearrange("(k p) m -> p k m", p=128)
            nc.sync.dma_start(out=xT_sb[:, :, 0:512], in_=xr[:, :, 0:512])
            nc.sync.dma_start(out=wq_sb[:, 4:KCH, :], in_=wqr[:, 4:KCH, :])
            cs_sb = wp.tile([128, ST, 128], F16, tag="cs", name="cs")
            nc.sync.dma_start(out=cs_sb[:], in_=cs2)
            triu_sb = wp.tile([128, 128], mm_dt, tag="triu", name="triu")
            nc.sync.dma_start(out=triu_sb[:], in_=triu)
            wkv_sb = wp.tile([128, KCH, 128], mm_dt, tag="wkv", name="wkv")
            nc.sync.dma_start(out=wkv_sb[:], in_=wkv.rearrange("(k p) m -> p k m", p=128))
            nc.sync.dma_start(out=xT_sb[:, :, 512:1024], in_=xr[:, :, 512:1024])
            nc.sync.dma_start(out=xT_sb[:, :, 1024:s], in_=xr[:, :, 1024:s])
            wo_sb = wp.tile([128, 4, HID], mm_dt, tag="wo", name="wo")
            for cc in range(4):
                w = 128 if cc < 3 else 64
                nc.sync.dma_start(out=wo_sb[0:w, cc, :], in_=wo[cc * 128:cc * 128 + w, :])

            idn = wp.tile([128, 128], F32, tag="idn", name="idn")
            make_identity(nc, idn[:])
            idn_r = wp.tile([128, 128], mm_dt, tag="idnr", name="idnr")
            nc.vector.tensor_copy(idn_r[:], idn[:])
            idn_mm = idn_r[:]
            # preload the exp table set while ACT is otherwise idle, so the
            # ~1.3us ACT_TABLE_LOAD is off the first real exp's critical path
            warm = wp.tile([1, 2], F32, tag="warm", name="warm")
            nc.vector.memset(warm[:], 0.0)
            nc.scalar.activation(out=warm[:], in_=warm[:], func=AF.Exp)
            # warm the PE clock (HAM un-throttles after ~3.4us of sustained
            # matmul activity) with dummy matmuls while the input DMAs are in
            # flight, so the first projections run at full rate
            with tc.tile_pool(name="psW", bufs=1, space="PSUM") as psW:
                warm_ps = psW.tile([128, 128], F32, tag="wps", name="wps")
                for _ in range(42):
                    nc.tensor.matmul(warm_ps[:], idn_r[:], idn_r[:],
                                     start=True, stop=True)

            # qT / aoT: qT in head-pair chunks (chunk j holds heads 2j, 2j+1);
            # aoT unified [128, 4, s] so the normalize muls batch across heads
            # with one strided AP per partition half.
            q_pair = [per.tile([128 if j < 3 else 64, s], mm_dt,
                               tag=f"qp{j}", name=f"qp{j}") for j in range(4)]
            aoT = per.tile([128, 4, s], mm_dt, tag="ao", name="ao")
            kT2 = per.tile([128, s], mm_dt, tag="kT2", name="kT2")
            # merged kv staging: [k(64) | v(64) | ones(1)] per s-tile
            kv_all = per.tile([128, ST, 129], mm_dt, tag="kv_all", name="kv_all")
            if mm_dt == F16:
                nc.vector.memset(
                    kv_all[:, :, 128:129].bitcast(mybir.dt.uint16), 0x3C00)
            else:
                nc.vector.memset(kv_all[:, :, 128:129].bitcast(F32), 1.0)

            def _body():
                LOOKP = 2
                with tc.tile_pool(name="psA", bufs=1, space="PSUM") as psA, \
                     tc.tile_pool(name="psQ", bufs=1, space="PSUM") as psQ:
                    # t-slot provider: one PSUM bank of 8 rotating [128,128]
                    # f16 quarter-slots for PE transposes.
                    tstate = {"slot": 0}
                    t_bank = psA.tile([128, 8, 128], mm_dt, tag="t", name="t")

                    def _tslot():
                        sl = tstate["slot"]
                        tstate["slot"] = (sl + 1) % 8
                        return t_bank[:, sl, :]

                    def _emit_qtrans(st, q_rot, pre=False):
                        # PSUM evacuations: ACT during the preamble (it is
                        # idle until the first exp), DVE in steady state
                        # (ACT is the kernel's critical engine there)
                        for cc in range(4):
                            w = 128 if cc < 3 else 64
                            t_ps = _tslot()[0:w, :]
                            nc.tensor.transpose(t_ps, q_rot[:, cc * 128:cc * 128 + w],
                                                idn_mm)
                            dst = q_pair[cc][:, st * 128:(st + 1) * 128]
                            if pre:
                                nc.scalar.copy(out=dst, in_=t_ps)
                            else:
                                nc.vector.tensor_copy(dst, t_ps)

                    pend_a = []
                    pend_mul = []
                    attn_state = {"psB": None}

                    def emit_out_st(st):
                        # one s-tile of y = ao @ Wo, emitted as a fill INSIDE
                        # the next block's attention. The y accumulator
                        # borrows a psB rotation slot ([128,1024] = same
                        # shape), so no extra PSUM banks are needed; the exp
                        # pipeline simply runs one buffer short for the ~1.5us
                        # the wo matmuls occupy the PE anyway.
                        y_ps = attn_state["psB"].tile([128, 1024], F32,
                                                      tag="sp", name="sp", bufs=2)
                        for cc in range(4):
                            w = 128 if cc < 3 else 64
                            lhsT = aoT[0:w, cc, st * 128:(st + 1) * 128]
                            nc.tensor.matmul(y_ps[:, 0:512], lhsT,
                                             wo_sb[0:w, cc, 0:512],
                                             start=(cc == 0), stop=(cc == 3))
                            nc.tensor.matmul(y_ps[:, 512:896], lhsT,
                                             wo_sb[0:w, cc, 512:896],
                                             start=(cc == 0), stop=(cc == 3))
                        y_sb = tmp.tile([128, HID], F32, tag="ysb", name="ysb")
                        nc.vector.tensor_copy(y_sb[:], y_ps[:, 0:896])
                        nc.sync.dma_start(out=y[st * 128:(st + 1) * 128, :],
                                          in_=y_sb[:])

                    def emit_proj(st, qpool=None, qbufs=1, pre=False):
                        if qpool is not None:
                            q_ps = qpool.tile([128, GD], F32, tag="q", name="q",
                                              bufs=qbufs)[:]
                            kv_ps = qpool.tile([128, 128], F32, tag="kv",
                                               name="kv", bufs=qbufs)[:]
                        else:
                            q_ps = psQ.tile([128, GD], F32, tag="q",
                                            name="q", bufs=1)[:]
                            kv_ps = psQ.tile([128, 128], F32, tag="kv",
                                             name="kv", bufs=1)[:]
                        for kc in range(KCH):
                            lhsT = xT_sb[:, kc, st * 128:(st + 1) * 128]
                            nc.tensor.matmul(q_ps, lhsT, wq_sb[:, kc, :],
                                             start=(kc == 0), stop=(kc == KCH - 1))
                        for kc in range(KCH):
                            lhsT = xT_sb[:, kc, st * 128:(st + 1) * 128]
                            nc.tensor.matmul(kv_ps, lhsT, wkv_sb[:, kc, :],
                                             start=(kc == 0), stop=(kc == KCH - 1))
                        # k, v staged raw (RoPE'd in place per block); q cast
                        # to f16 for the 2x-mode RoPE chain
                        q_sb = tmp.tile([128, GD], F16, tag="qsb", name="qsb",
                                        bufs=2)
                        if pre:
                            nc.scalar.copy(out=kv_all[:, st, 0:128], in_=kv_ps)
                            nc.scalar.copy(out=q_sb[:], in_=q_ps)
                        else:
                            nc.vector.tensor_copy(kv_all[:, st, 0:128], kv_ps)
                            nc.vector.tensor_copy(q_sb[:], q_ps)
                        # rotate-half RoPE, all-f16 contiguous 32-blocks (2x):
                        #   qrot[:32] = q[:32]*c - q[32:]*s
                        #   qrot[32:] = q[:32]*s + q[32:]*c
                        # via qrot = q*cos2 + swap(q)*sin2, sin2 = [-s | s].
                        qv = q_sb[:].rearrange("p (h d) -> p h d", d=D)
                        cb = _bc7(cs_sb[:, st, 0:64])
                        sb_ = _bc7(cs_sb[:, st, 64:128])
                        t1 = tmp.tile([128, HL, D], F16, tag="t1", name="t1", bufs=2)
                        t2 = tmp.tile([128, HL, D], F16, tag="t2", name="t2", bufs=2)
                        nc.vector.tensor_mul(t1[:], qv, cb)
                        nc.vector.tensor_mul(t2[:], _swap_halves(qv), sb_)
                        q_rot = tmp.tile([128, GD], mm_dt, tag="qrot", name="qrot",
                                         bufs=3)
                        qrv = q_rot[:].rearrange("p (h d) -> p h d", d=D)
                        nc.vector.tensor_add(qrv, t1[:], t2[:])
                        # transposes for the PREVIOUS s-tile go after this
                        # tile's projections so PE never waits on the RoPE DVE
                        pend_a.append((st, q_rot))
                        if len(pend_a) > 1:
                            _emit_qtrans(*pend_a.pop(0), pre=pre)

                    def emit_krope(st_lo, st_hi):
                        n = st_hi - st_lo
                        kv3 = kv_all[:, st_lo:st_hi, 0:64]
                        cs = cs_sb[:, st_lo:st_hi, 0:64]
                        ss = cs_sb[:, st_lo:st_hi, 64:128]
                        k1 = tmp.tile([128, TPB, 64], F16, tag="k1", name="k1", bufs=2)
                        k2 = tmp.tile([128, TPB, 64], F16, tag="k2", name="k2", bufs=2)
                        nc.vector.tensor_mul(k1[:, 0:n, :], kv3, cs)
                        nc.vector.tensor_mul(k2[:, 0:n, :], _swap_halves(kv3), ss)
                        nc.vector.tensor_add(kv3, k1[:, 0:n, :], k2[:, 0:n, :])

                    def emit_ktrans(st_lo, st_hi, pre=False):
                        for st in range(st_lo, st_hi):
                            t_ps = _tslot()[0:64, :]
                            nc.tensor.transpose(t_ps, kv_all[:, st, 0:64], idn_mm)
                            lo = kT2[0:64, st * 128:(st + 1) * 128]
                            hi = kT2[64:128, st * 128:(st + 1) * 128]
                            if pre:
                                nc.scalar.copy(out=lo, in_=t_ps)
                                nc.scalar.copy(out=hi, in_=t_ps)
                            else:
                                nc.vector.tensor_copy(lo, t_ps)
                                nc.vector.tensor_copy(hi, t_ps)

                    def emit_attn(qc, fills=()):
                        # k-tiles in pairs sharing a [128,1024] psum + one exp;
                        # LOOKP pairs in flight so PE stays ahead of ACT.
                        # Unwritten psum regions of partial (diagonal) tiles
                        # hold stale garbage whose exp is never consumed.
                        araw = rb.tile([65, HL, 512], F32, tag="araw",
                                       name="araw", bufs=2)
                        rbF = rb.tile([64, HL, 512], F32, tag="rbF",
                                      name="rbF", bufs=2)
                        with tc.tile_pool(name="psB", bufs=1, space="PSUM") as psB, \
                             tc.tile_pool(name="psO", bufs=1, space="PSUM") as psO:
                            attn_state["psB"] = psB
                            nkt = 4 * (qc + 1)
                            npair = (nkt + 1) // 2
                            fill_iter = iter(fills)

                            def emit_pair(h, pi, qc=qc, nkt=nkt):
                                half = (h % 2) * 64
                                qsrc = q_pair[h // 2]
                                s_ps = psB.tile([128, 1024], F32, tag="sp",
                                                name="sp", bufs=2)
                                ex = expp.tile([128, 1024], mm_dt, tag="ex",
                                               name="ex", bufs=4)
                                info = []
                                for j in (0, 1):
                                    kt = 2 * pi + j
                                    if kt >= nkt:
                                        break
                                    rrel = kt - 4 * qc
                                    off = 128 * rrel if rrel >= 0 else 0
                                    N = 512 - off
                                    nc.tensor.matmul(
                                        s_ps[:, 512 * j + off:512 * (j + 1)],
                                        kT2[half:half + 64, kt * 128:(kt + 1) * 128],
                                        qsrc[half:half + 64,
                                             qc * 512 + off:(qc + 1) * 512],
                                        start=True, stop=True)
                                    info.append((kt, 512 * j + off, off, N, rrel))
                                # exp over each contiguous written run (a
                                # diagonal second tile leaves an unwritten gap)
                                runs = []
                                for kt, base, off, N, rrel in info:
                                    if runs and runs[-1][1] == base:
                                        runs[-1][1] = base + N
                                    else:
                                        runs.append([base, base + N])
                                for lo, hi in runs:
                                    nc.scalar.activation(out=ex[:, lo:hi],
                                                         in_=s_ps[:, lo:hi],
                                                         func=AF.Exp)
                                for kt, base, off, N, rrel in info:
                                    if rrel >= 0:
                                        nc.vector.tensor_mul(
                                            ex[:, base:base + 128],
                                            ex[:, base:base + 128], triu_sb[:])
                                return ex, info

                            # flat (head, pair) task stream: the scores/exp
                            # lookahead crosses head boundaries, so ACT keeps
                            # an exp backlog through evacuations and fills.
                            tasks = [(h, pi) for h in range(HL)
                                     for pi in range(npair)]
                            LOOKA = LOOKP + 1
                            pend = {}
                            o_ref = {}
                            for i in range(min(LOOKA, len(tasks))):
                                pend[tasks[i]] = emit_pair(*tasks[i])
                            for i, (h, pi) in enumerate(tasks):
                                if i + LOOKA < len(tasks):
                                    pend[tasks[i + LOOKA]] = emit_pair(*tasks[i + LOOKA])
                                if pi == 0:
                                    o_ref[h] = psO.tile([65, 512], F32, tag="o",
                                                        name="o", bufs=1)
                                o_ps = o_ref[h]
                                ex, info = pend.pop((h, pi))
                                for kt, base, off, N, rrel in info:
                                    nc.tensor.matmul(
                                        o_ps[:, off:512], kv_all[:, kt, 64:129],
                                        ex[:, base:base + N],
                                        start=(kt == 0), stop=(kt == nkt - 1))
                                if pi == npair - 1:
                                    # ---- per-head-chunk softmax norm ----
                                    # attn rows + den row staged in one copy
                                    # (frees the psum fast); den row then
                                    # partition-broadcast on Pool (SBUF-only
                                    # engine); fast-reciprocal of the
                                    # broadcast rows; the normalize multiply
                                    # is DEFERRED a block and batched across
                                    # heads.
                                    nc.vector.tensor_copy(araw[:, h, :],
                                                          o_ps[:])
                                    # den row staged to a partition-0 tile:
                                    # the Pool broadcast reads partition 0
                                    den_sb = tmp.tile([1, 512], F32, tag="den",
                                                      name="den", bufs=4)
                                    nc.vector.tensor_copy(den_sb[:],
                                                          o_ps[64:65, :])
                                    denB = tmp.tile([64, 512], F32, tag="denB",
                                                    name="denB", bufs=4)
                                    nc.gpsimd.partition_broadcast(denB[:],
                                                                  den_sb[:])
                                    nc.vector.reciprocal_approx_fast(
                                        rbF[:, h, :], denB[:])
                                    # block 0: defer fills past head 1 so the
                                    # list scheduler can't run next-block
                                    # projections ahead of this block's first
                                    # scores (which wait on the preamble's
                                    # transpose chain)
                                    if qc > 0 or h >= 2:
                                        for _ in range(2):
                                            f = next(fill_iter, None)
                                            if f is not None:
                                                f()
                        pend_mul.append((qc, araw[:], rbF[:]))

                    def _emit_muls(qc, araw_ap, rbF_ap, fd_off=0, fd_len=512):
                        # heads 0,2,4,6 -> aoT partitions 0:64, chunks 0..3;
                        # heads 1,3,5 -> partitions 64:128, chunks 0..2.
                        ao = aoT[:]
                        dst0 = bass.AP(
                            tensor=ao.tensor,
                            offset=ao.offset + qc * 512 + fd_off,
                            ap=[[ao.ap[0][0], 64], [ao.ap[1][0], 4], [1, fd_len]])
                        nc.vector.tensor_mul(
                            dst0, _hsel(araw_ap, 0, 2, 4, fd_off, fd_len),
                            _hsel(rbF_ap, 0, 2, 4, fd_off, fd_len))
                        dst1 = bass.AP(
                            tensor=ao.tensor,
                            offset=ao.offset + 64 * ao.ap[0][0] + qc * 512 + fd_off,
                            ap=[[ao.ap[0][0], 64], [ao.ap[1][0], 3], [1, fd_len]])
                        nc.vector.tensor_mul(
                            dst1, _hsel(araw_ap, 1, 2, 3, fd_off, fd_len),
                            _hsel(rbF_ap, 1, 2, 3, fd_off, fd_len))

                    def _flush_muls():
                        for args in pend_mul:
                            _emit_muls(*args)
                        pend_mul.clear()

                    def emit_out_one(st, pool, tail=False):
                        y_ps = pool.tile([128, 1024], F32, tag="y", name="y")
                        for cc in range(4):
                            w = 128 if cc < 3 else 64
                            lhsT = aoT[0:w, cc, st * 128:(st + 1) * 128]
                            nc.tensor.matmul(y_ps[:, 0:512], lhsT,
                                             wo_sb[0:w, cc, 0:512],
                                             start=(cc == 0), stop=(cc == 3))
                            nc.tensor.matmul(y_ps[:, 512:896], lhsT,
                                             wo_sb[0:w, cc, 512:896],
                                             start=(cc == 0), stop=(cc == 3))
                        y_sb = tmp.tile([128, HID], F32, tag="ysb", name="ysb")
                        if tail:
                            nc.scalar.copy(out=y_sb[:], in_=y_ps[:, 0:896])
                        else:
                            nc.vector.tensor_copy(y_sb[:], y_ps[:, 0:896])
                        nc.sync.dma_start(out=y[st * 128:(st + 1) * 128, :],
                                          in_=y_sb[:])

                    # fine-grained interleave: the next block's projections,
                    # k-RoPE, transposes and the previous block's deferred
                    # normalization muls are emitted BETWEEN attention chunks,
                    # so the PE queue always holds independent matmuls where
                    # the exp's fixed per-instruction overhead would otherwise
                    # idle it.
                    def _mk_flushn():
                        n0 = len(pend_mul)

                        def _f():
                            for args in pend_mul[:n0]:
                                _emit_muls(*args)
                            del pend_mul[:n0]
                        return _f

                    def _mk_fills(nb):
                        # fills for emit_attn(nb-1): flush block nb-2's
                        # normalize muls, then its 4 wo s-tiles interleaved
                        # with block nb's 4 projections, then k-RoPE and the
                        # transposes for block nb.
                        # steady-state fill evacuations stay on DVE; ACT only
                        # absorbs them in the preamble/tail where it idles.
                        # Transposes are spread across per-tile kq fills so
                        # the 6 packed t-slots never back up.
                        def _kq(st):
                            def _f():
                                emit_krope(st, st + 1)
                                while pend_a and pend_a[0][0] <= st:
                                    _emit_qtrans(*pend_a.pop(0))
                                emit_ktrans(st, st + 1)
                            return _f

                        fills = [_mk_flushn()] if nb >= 2 else []
                        for i in range(TPB):
                            if nb >= 2:
                                fills.append(lambda t=(nb - 2) * TPB + i: emit_out_st(t))
                            if nb < QC:
                                fills.append(lambda t=nb * TPB + i: emit_proj(t))
                                if i >= 1:
                                    fills.append(_kq(nb * TPB + i - 1))
                        if nb < QC:
                            fills.append(_kq(nb * TPB + TPB - 1))
                        return fills

                    # preamble: per-s-tile proj -> RoPE -> transposes, fully
                    # pipelined so the first scores/exp start as early as
                    # possible (the first chunk's scores need ALL 4 s-tiles'
                    # qT, so each tile's transposes overlap the next tile's
                    # projection matmuls). A dedicated double-buffered PSUM
                    # pool (banks are otherwise idle before attention starts)
                    # keeps the projections back-to-back on the PE instead of
                    # serializing on each tile's evacuation.
                    with tc.tile_pool(name="psP", bufs=1, space="PSUM") as psP:
                        emit_proj(0, psP, 2, pre=True)
                        for st in range(1, TPB):
                            emit_proj(st, psP, 2, pre=True)
                            emit_krope(st - 1, st)
                            while pend_a and pend_a[0][0] < st:
                                _emit_qtrans(*pend_a.pop(0), pre=True)
                            emit_ktrans(st - 1, st, pre=True)
                        emit_krope(TPB - 1, TPB)
                        while pend_a:
                            _emit_qtrans(*pend_a.pop(0), pre=True)
                        emit_ktrans(TPB - 1, TPB, pre=True)
                    for blk in range(QC):
                        emit_attn(blk, _mk_fills(blk + 1))
                    # tail: the last block's normalize muls are split per
                    # s-tile and interleaved with its wo projections, so the
                    # PE starts on y as soon as the first 128 queries are
                    # normalized instead of after the full batched muls.
                    (qcL, arawL, rbFL) = pend_mul.pop()
                    assert not pend_mul
                    with tc.tile_pool(name="psC", bufs=2, space="PSUM") as psC:
                        for i in range(TPB):
                            _emit_muls(qcL, arawL, rbFL, i * 128, 128)
                            emit_out_one(qcL * TPB + i, psC, tail=True)

            if reps > 1:
                with tc.For_i(0, reps, 1):
                    _body()
            else:
                _body()

    nc.compile()
    return nc


# ---------------------------------------------------------------------------
# host-side sharding + execution
# ---------------------------------------------------------------------------

def round_f32r(a):
    """Round fp32 array to fp32r (RNE to 11 mantissa bits)."""
    b = np.ascontiguousarray(a, dtype=np.float32).view(np.uint32)
    lsb = (b >> np.uint32(12)) & np.uint32(1)
    r = ((b + np.uint32(0x7FF) + lsb) & np.uint32(0xFFFFF000))
    return r.view(np.float32)


MM_DT = {"f32r": F32R, "f16": F16, "f32": F32}[os.environ.get("MM_DT", "f16")]


def _cvt(a, mm_dt):
    if mm_dt == F16:
        return np.ascontiguousarray(np.asarray(a, dtype=np.float32)).astype(np.float16)
    if mm_dt == F32R:
        return round_f32r(a)
    return np.ascontiguousarray(a, dtype=np.float32)


# rotate-half channel permutation within each 64-channel head: evens first,
# then odds. Applied to Wq/Wk columns; scores are invariant to a shared
# permutation of q and k channels, and v/Wo are untouched.
_PERM64 = np.concatenate([np.arange(0, 64, 2), np.arange(1, 64, 2)])


def _perm_heads(w, nheads):
    w = np.asarray(w)
    cols = np.concatenate([h * 64 + _PERM64 for h in range(nheads)])
    return w[:, cols]


def make_in_maps(x, freqs_cos, freqs_sin, Wq, Wk, Wv, Wo, s=S, mm_dt=None):
    if mm_dt is None:
        mm_dt = MM_DT
    ST = s // 128
    scale = 1.0 / math.sqrt(D)
    cosr = np.ascontiguousarray(
        np.asarray(freqs_cos).reshape(ST, 128, 32).transpose(1, 0, 2)).astype(np.float32)
    sinr = np.ascontiguousarray(
        np.asarray(freqs_sin).reshape(ST, 128, 32).transpose(1, 0, 2)).astype(np.float32)
    cs2 = np.concatenate([cosr, cosr, -sinr, sinr], axis=2).astype(np.float16)
    triu = _cvt(np.triu(np.ones((128, 128), dtype=np.float32)), mm_dt)
    in_maps = []
    for c in range(N_CORES):
        b, g = c // 2, c % 2
        wq_g = _perm_heads(np.asarray(Wq)[:, g * GD:(g + 1) * GD] * scale, HL)
        wk_g = _perm_heads(np.asarray(Wk)[:, g * D:(g + 1) * D], 1)
        in_maps.append({
            "xT": _cvt(np.asarray(x)[b].T, mm_dt),
            "wq": _cvt(wq_g, mm_dt),
            "wkv": _cvt(np.concatenate(
                [wk_g, np.asarray(Wv)[:, g * D:(g + 1) * D]], axis=1), mm_dt),
            "wo": _cvt(np.asarray(Wo)[g * GD:(g + 1) * GD, :], mm_dt),
            "cs2": cs2, "triu": triu,
        })
    return in_maps


_RUNNER = None


class _Runner:
    """Minimal SPMD executor over axon PJRT (self-contained copy)."""

    def __init__(self, nc, n_cores):
        import jax
        from jax.sharding import Mesh, PartitionSpec, NamedSharding
        from jax.experimental.shard_map import shard_map
        from concourse.bass2jax import (_bass_exec_p, install_neuronx_cc_hook,
                                        partition_id_tensor)
        install_neuronx_cc_hook()
        self.jax = jax
        self.n_cores = n_cores
        partition_name = (nc.partition_id_tensor.name
                          if nc.partition_id_tensor else None)
        in_names, out_names, out_avals = [], [], []
        for alloc in nc.m.functions[0].allocations:
            if not isinstance(alloc, mybir.MemoryLocationSet):
                continue
            name = alloc.memorylocations[0].name
            if alloc.kind == "ExternalInput":
                if name != partition_name:
                    in_names.append(name)
            elif alloc.kind == "ExternalOutput":
                out_names.append(name)
                out_avals.append(jax.core.ShapedArray(
                    tuple(alloc.tensor_shape), mybir.dt.np(alloc.dtype)))
        self.in_names, self.out_names, self.out_avals = in_names, out_names, out_avals
        n_params, n_outs = len(in_names), len(out_avals)
        all_names = in_names + out_names
        if partition_name is not None:
            all_names.append(partition_name)

        def _body(*args):
            operands = list(args)
            if partition_name is not None:
                operands.append(partition_id_tensor())
            return tuple(_bass_exec_p.bind(
                *operands, out_avals=tuple(out_avals), in_names=tuple(all_names),
                out_names=tuple(out_names), lowering_input_output_aliases=(),
                sim_require_finite=False, sim_require_nnan=False, nc=nc))

        devices = jax.devices()[:n_cores]
        self.mesh = Mesh(np.asarray(devices), ("core",))
        self.sharding = NamedSharding(self.mesh, PartitionSpec("core"))
        in_specs = (PartitionSpec("core"),) * (n_params + n_outs)
        out_specs = (PartitionSpec("core"),) * n_outs
        self.fn = jax.jit(
            shard_map(_body, mesh=self.mesh, in_specs=in_specs,
                      out_specs=out_specs, check_rep=False),
            donate_argnums=tuple(range(n_params, n_params + n_outs)),
            keep_unused=True)
        zshapes = [(n_cores * a.shape[0], *a.shape[1:]) for a in out_avals]
        zdtypes = [a.dtype for a in out_avals]
        self.make_zeros = jax.jit(
            lambda: tuple(jax.numpy.zeros(sh, dt)
                          for sh, dt in zip(zshapes, zdtypes)),
            out_shardings=tuple(self.sharding for _ in zshapes))

    def prep(self, in_maps):
        return [self.jax.device_put(
            np.concatenate([np.asarray(in_maps[c][n]) for c in range(self.n_cores)],
                           axis=0), self.sharding)
            for n in self.in_names]

    def run(self, dev_in):
        return self.fn(*dev_in, *self.make_zeros())

    def split(self, outs):
        res = []
        for c in range(self.n_cores):
            res.append({n: np.asarray(outs[i]).reshape(
                self.n_cores, *self.out_avals[i].shape)[c]
                for i, n in enumerate(self.out_names)})
        return res


def get_runner():
    global _RUNNER
    if _RUNNER is None:
        _RUNNER = _Runner(build(), N_CORES)
    return _RUNNER


def kernel(x, freqs_cos, freqs_sin, mask, Wq, Wk, Wv, Wo):
    x = np.asarray(x, dtype=np.float32)
    in_maps = make_in_maps(np.asarray(x), np.asarray(freqs_cos),
                           np.asarray(freqs_sin), np.asarray(Wq),
                           np.asarray(Wk), np.asarray(Wv), np.asarray(Wo))
    r = get_runner()
    outs = r.run(r.prep(in_maps))
    res = r.split(outs)
    out = np.empty((B, S, HID), dtype=np.float32)
    for b in range(B):
        out[b] = res[2 * b]["y"] + res[2 * b + 1]["y"]
    return out
